# revision 1
# baseline (speedup 1.0000x reference)
"""Trainium2 Bass kernel for CausalSelfAttention (d_model=2048, 16 heads, s=2048, b=2).

Sharding: data-parallel over batch (2) x tensor-parallel over heads (4 groups
of 4 heads) = 8 cores. Each core computes qkv projection for its 4 heads on its
batch, RoPE, causal attention, and a partial o_proj (row-parallel over the
head dimension). Host sums the 4 partial outputs per batch.

All matmuls run in float32r (TF32-like, full PE rate); softmax in fp32.

Layouts (per core):
  x_T   [dm, s]   activations, feature-major (host-pre-transposed)
  q/k   [dh, s]   per head, feature-major -> scores_T = k_tile^T @ q_chunk
  p_T   [k,  q]   exp(scores_T) -- softmax without max-subtraction (bounded
                  scores; constant shift -25 applied in the exp bias)
  attn_T [dh, q]  = v_tile^T @ p_T  (v in natural [s, dh] is exactly lhsT)
  out_T [dm, s]   partial o_proj output (host transposes + sums over groups)

Head 0's q/k/v bypass the DRAM scratch staging through persistent SBUF tiles
so attention starts as soon as the projection finishes.
"""

import sys

import numpy as np

_TRN_REPO = "/opt/trn_rl_repo"
if _TRN_REPO not in sys.path:
    sys.path.insert(0, _TRN_REPO)

import concourse.tile as tile  # noqa: E402
import concourse.mybir as mybir  # noqa: E402
from concourse import bacc, bass_utils  # noqa: E402

# Problem constants (hardcoded per the contract).
S = 2048          # sequence length
B = 2             # batch
DM = 2048         # d_model
NH = 16           # heads total
DH = 128          # head dim
ROPE_THETA = 10000.0

N_CORES = 8
TP = 4            # head-parallel groups
HPC = NH // TP    # heads per core = 4
DHC = HPC * DH    # head-dim per core = 512

SC = 512          # s-chunk (matmul moving dim)
NSC = S // SC     # 4 chunks
KT = DM // 128    # contraction tiles for projections = 16
NVT = S // 128    # v tiles per head = 16

SCALE = 1.0 / float(np.sqrt(DH))
EXP_SHIFT = -25.0  # softmax computed as exp(score*scale - 25); shift cancels

F32 = mybir.dt.float32
F32R = mybir.dt.float32r

WARMUP_MMS = 130

_CACHE = {}


def _build_program():
    nc = bacc.Bacc("TRN2", target_bir_lowering=False, debug=False,
                   num_devices=N_CORES)

    # ---- I/O ----
    x_T = nc.dram_tensor("x_T", [DM, S], F32R, kind="ExternalInput")
    wq_T = nc.dram_tensor("wq_T", [DM, DHC], F32R, kind="ExternalInput")
    wk_T = nc.dram_tensor("wk_T", [DM, DHC], F32R, kind="ExternalInput")
    wv_T = nc.dram_tensor("wv_T", [DM, DHC], F32R, kind="ExternalInput")
    wo_T = nc.dram_tensor("wo_T", [DHC, DM], F32R, kind="ExternalInput")
    cos_t = nc.dram_tensor("cos_t", [DH, S], F32, kind="ExternalInput")
    sin_t = nc.dram_tensor("sin_t", [DH, S], F32, kind="ExternalInput")
    mask_wide = nc.dram_tensor("mask_wide", [128, 384 + SC], F32,
                               kind="ExternalInput")
    ones_col = nc.dram_tensor("ones_col", [128, 1], F32R, kind="ExternalInput")
    out_T = nc.dram_tensor("out_T", [DM, S], F32, kind="ExternalOutput")

    with tile.TileContext(nc) as tc:
      with (
          tc.tile_pool(name="dram", bufs=1, space="DRAM") as dpool,
          tc.tile_pool(name="bridge", bufs=1) as bpool,
      ):
        # DRAM scratch for staged q/k/v (heads 1..3; head 0 stays in SBUF).
        q_sc = [None] + [dpool.tile([DH, S], F32R, tag=f"qsc{h}", name=f"qsc{h}")
                         for h in range(1, HPC)]
        k_sc = [None] + [dpool.tile([DH, S], F32R, tag=f"ksc{h}", name=f"ksc{h}")
                         for h in range(1, HPC)]
        v_sc = [None] + [dpool.tile([S, DH], F32R, tag=f"vsc{h}", name=f"vsc{h}")
                         for h in range(1, HPC)]
        # Head-0 SBUF bridge.
        qh0 = bpool.tile([DH, S], F32R, tag="qh0")
        kh0 = bpool.tile([DH, S], F32R, tag="kh0")
        vh0 = [bpool.tile([128, DH], F32R, tag=f"vh0_{i}", name=f"vh0_{i}")
               for i in range(NVT)]
        # ================= Phase A: QKV projection + RoPE =================
        with (
            tc.tile_pool(name="wqkv", bufs=1) as wpool,
            tc.tile_pool(name="xin", bufs=3) as xpool,
            tc.tile_pool(name="csin", bufs=1) as cpool,
            tc.tile_pool(name="evac", bufs=2) as epool,
            tc.tile_pool(name="psA", bufs=7, space="PSUM") as psA,
        ):
            # PE warmup: the first weight/activation DMAs take ~14us; keep the
            # PE busy with zero matmuls meanwhile so HAM is at full clock when
            # the real accumulations start (idle >3.4us re-throttles it).
            wu_d = cpool.tile([128, SC], F32, tag="wud")
            nc.gpsimd.memset(wu_d[:], 0.0)
            wu_ps = psA.tile([128, SC], F32, tag="wu", bufs=1)

            def _warmup(n, lhs):
                for _ in range(n):
                    nc.tensor.matmul(wu_ps[:, 0:32], lhs, wu_d[:, 0:32],
                                     start=True, stop=True)
            # Weights as merged wide tiles: one DMA each via a strided DRAM
            # access pattern (HWDGE fixed overhead is per-DMA, so batch).
            HKT = KT // 2
            wq_w = [wpool.tile([128, HKT * DHC], F32R, tag=f"wqw{i}",
                               name=f"wqw{i}") for i in range(2)]
            wk_w = wpool.tile([128, KT * DHC], F32R, tag="wkw")
            wv_w = wpool.tile([128, KT * DHC], F32R, tag="wvw")
            cos_sb = cpool.tile([DH, S], F32, tag="cos")
            sin_sb = cpool.tile([DH, S], F32, tag="sin")

            def _w_src(t, i0, n):
                # [dk*128+p, c] -> [p, dk, c] for dk in [i0, i0+n)
                return t[:, :].rearrange("(dk p) c -> p dk c", p=128)[:, i0:i0 + n, :]

            def _wq_ap(dk):
                return wq_w[dk // HKT][:, (dk % HKT) * DHC:(dk % HKT + 1) * DHC]

            def _wk_ap(dk):
                return wk_w[:, dk * DHC:(dk + 1) * DHC]

            def _wv_ap(dk):
                return wv_w[:, dk * DHC:(dk + 1) * DHC]

            # x chunks in merged half-chunk DMAs (8 dk-tiles per transfer).
            def _load_x_half(sc, i, name):
                xt = xpool.tile([128, HKT * SC], F32R, tag="x", name=name)
                src = (x_T[:, sc * SC:(sc + 1) * SC]
                       .rearrange("(dk p) s -> p dk s", p=128)[:, i * HKT:(i + 1) * HKT, :])
                nc.sync.dma_start(xt[:].rearrange("p (dk s) -> p dk s", s=SC), src)
                return xt

            # Startup order: first x half, first wq half, second x half, ...
            _warmup(WARMUP_MMS, wu_d[:, 0:128])
            x0a = _load_x_half(0, 0, "x0a")
            nc.sync.dma_start(
                wq_w[0][:].rearrange("p (dk c) -> p dk c", c=DHC),
                _w_src(wq_T, 0, HKT))
            x0b = _load_x_half(0, 1, "x0b")
            nc.sync.dma_start(
                wq_w[1][:].rearrange("p (dk c) -> p dk c", c=DHC),
                _w_src(wq_T, HKT, HKT))
            nc.sync.dma_start(cos_sb[:], cos_t[:, :])
            nc.sync.dma_start(sin_sb[:], sin_t[:, :])
            nc.sync.dma_start(
                wk_w[:].rearrange("p (dk c) -> p dk c", c=DHC),
                _w_src(wk_T, 0, KT))
            nc.sync.dma_start(
                wv_w[:].rearrange("p (dk c) -> p dk c", c=DHC),
                _w_src(wv_T, 0, KT))

            for sc in range(NSC):
                ssl = slice(sc * SC, (sc + 1) * SC)
                if sc == 0:
                    xh = [x0a, x0b]
                else:
                    xh = [_load_x_half(sc, 0, f"x{sc}a"),
                          _load_x_half(sc, 1, f"x{sc}b")]
                xc = [xh[dk // HKT][:, (dk % HKT) * SC:(dk % HKT + 1) * SC]
                      for dk in range(KT)]

                # q and k for each head, with RoPE.
                for which, wap in (("q", _wq_ap), ("k", _wk_ap)):
                    for h in range(HPC):
                        hsl = slice(h * DH, (h + 1) * DH)
                        ps = psA.tile([128, SC], F32, tag="mm")
                        for dk in range(KT):
                            nc.tensor.matmul(ps[:], wap(dk)[:, hsl], xc[dk],
                                             start=(dk == 0), stop=(dk == KT - 1))
                        # Evacuate to SBUF (f32r).  Host permuted the head
                        # dims so each RoPE pair sits 16 partitions apart
                        # within a 32-block: the swap is one DVE shuffle.
                        raw = epool.tile([128, SC], F32R, tag="raw")
                        nc.scalar.copy(raw[:], ps[:])
                        qsw = epool.tile([128, SC], F32, tag="qsw")
                        nc.vector.stream_shuffle(
                            qsw[:], raw[:].bitcast(F32),
                            mask=list(range(16, 32)) + list(range(0, 16)))
                        # rot = raw*cos2 + qsw*sin2   (sin2 carries the sign)
                        nc.vector.tensor_mul(qsw[:], qsw[:], sin_sb[:, ssl])
                        t1 = epool.tile([128, SC], F32, tag="t1")
                        nc.gpsimd.tensor_mul(t1[:], raw[:].bitcast(F32),
                                             cos_sb[:, ssl])
                        if h == 0:
                            dst_ap = (qh0 if which == "q" else kh0)[:, ssl]
                            nc.vector.tensor_add(dst_ap, qsw[:], t1[:])
                        else:
                            rot = epool.tile([128, SC], F32R, tag="rot")
                            nc.vector.tensor_add(rot[:], qsw[:], t1[:])
                            dst = (q_sc if which == "q" else k_sc)[h]
                            nc.sync.dma_start(dst[:, ssl], rot[:])

                # v: natural [s, d] layout.
                for st in range(SC // 128):
                    row = sc * SC + st * 128
                    ps = psA.tile([128, DHC], F32, tag="mm")
                    for dk in range(KT):
                        nc.tensor.matmul(
                            ps[:], xc[dk][:, st * 128:(st + 1) * 128],
                            _wv_ap(dk), start=(dk == 0), stop=(dk == KT - 1))
                    nc.scalar.copy(vh0[sc * 4 + st][:], ps[:, 0:DH])
                    vsb = epool.tile([128, DHC - DH], F32R, tag="vsb")
                    nc.scalar.copy(vsb[:], ps[:, DH:])
                    for h in range(1, HPC):
                        nc.sync.dma_start(
                            v_sc[h][row:row + 128, :],
                            vsb[:, (h - 1) * DH:h * DH])

        # ================= Phase B: attention =================
        with tc.tile_pool(name="anorm", bufs=1) as apool, \
             tc.tile_pool(name="wo", bufs=1) as wop:
          # Normalized attention outputs (consumed by phase C).
          attn_n = [apool.tile([DH, S], F32R, tag=f"an{h}", name=f"an{h}")
                    for h in range(HPC)]
          # o_proj weights prefetched mid-attention (after head-2 loads).
          wo_t = [wop.tile([128, DM], F32R, tag=f"wo{h}", name=f"wo{h}")
                  for h in range(HPC)]
          with (
            tc.tile_pool(name="qkvh", bufs=2) as hpool,
            tc.tile_pool(name="cst", bufs=1) as cstp,
            tc.tile_pool(name="ptile", bufs=10) as ppool,
            tc.tile_pool(name="small", bufs=4) as spool,
            tc.tile_pool(name="psS", bufs=5, space="PSUM") as psS,
            tc.tile_pool(name="psAcc", bufs=2, space="PSUM") as psAcc,
            tc.tile_pool(name="psDen", bufs=1, space="PSUM") as psDen,
        ):
            mask_sb = cstp.tile([128, 384 + SC], F32, tag="mask")
            nc.sync.dma_start(mask_sb[:], mask_wide[:, :])
            onec_sb = cstp.tile([128, 1], F32R, tag="onec")
            nc.sync.dma_start(onec_sb[:], ones_col[:, :])
            bias_sb = cstp.tile([128, 1], F32, tag="bias")
            nc.vector.memset(bias_sb[:], EXP_SHIFT)

            for h in range(HPC):
                if h == 0:
                    qh, kh = qh0, kh0
                    vh = [t[:] for t in vh0]
                else:
                    qh = hpool.tile([DH, S], F32R, tag="qh", name=f"qh{h}")
                    kh = hpool.tile([DH, S], F32R, tag="kh", name=f"kh{h}")
                    nc.sync.dma_start(qh[:], q_sc[h][:, :])
                    nc.sync.dma_start(kh[:], k_sc[h][:, :])
                    vh_all = hpool.tile([128, NVT * DH], F32R, tag="vha",
                                        name=f"vha{h}")
                    nc.sync.dma_start(
                        vh_all[:].rearrange("p (n d) -> p n d", d=DH),
                        v_sc[h][:, :].rearrange("(n p) d -> p n d", p=128))
                    vh = [vh_all[:, kt * DH:(kt + 1) * DH] for kt in range(NVT)]
                    if h == 2:
                        for hh in range(HPC):
                            nc.sync.dma_start(wo_t[hh][:],
                                              wo_T[hh * 128:(hh + 1) * 128, :])

                for qc in range(NSC):
                    qsl = slice(qc * SC, (qc + 1) * SC)
                    n_kt = 4 * qc + 4  # causal: only k tiles with 128*kt < 512*(qc+1)
                    acc = psAcc.tile([128, SC], F32, tag="acc")
                    den = psDen.tile([1, SC], F32, tag="den")
                    for kt in range(n_kt):
                        off = kt * 128 - qc * SC
                        last = kt == n_kt - 1
                        # Diagonal tiles: columns q_rel < off are fully above
                        # the causal boundary, so compute only [q_lo, 512)
                        # (f32r needs a moving dim >= 256, hence q_lo caps at
                        # 256).  Within the live region only the first `mc`
                        # columns can contain masked elements.
                        if off <= 0:
                            q_lo, mc, oe = 0, (128 if off == 0 else 0), 0
                        elif off == 128:
                            q_lo, mc, oe = 128, 128, 0
                        elif off == 256:
                            q_lo, mc, oe = 256, 128, 0
                        else:  # off == 384
                            q_lo, mc, oe = 256, 256, 128
                        ln = SC - q_lo
                        sp = psS.tile([128, SC], F32, tag="sc")
                        nc.tensor.matmul(
                            sp[:, 0:ln], kh[:, kt * 128:(kt + 1) * 128],
                            qh[:, qc * SC + q_lo:(qc + 1) * SC],
                            start=True, stop=True)
                        pt = ppool.tile([128, SC], F32R, tag="pt")
                        nc.scalar.activation(
                            pt[:, 0:ln], sp[:, 0:ln],
                            mybir.ActivationFunctionType.Exp,
                            bias=bias_sb[:], scale=SCALE)
                        if mc:
                            nc.vector.tensor_mul(
                                pt[:, 0:mc], pt[:, 0:mc].bitcast(F32),
                                mask_sb[:, 384 - oe:384 - oe + mc])
                        nc.tensor.matmul(den[:, q_lo:SC], onec_sb[:],
                                         pt[:, 0:ln],
                                         start=(kt == 0), stop=last)
                        nc.tensor.matmul(acc[:, q_lo:SC], vh[kt],
                                         pt[:, 0:ln],
                                         start=(kt == 0), stop=last)
                    recipf = spool.tile([1, SC], F32, tag="recipf")
                    nc.vector.reciprocal_approx_fast(out=recipf[:], in_=den[:])
                    rbs = spool.tile([128, SC], F32, tag="rbs")
                    nc.gpsimd.partition_broadcast(rbs[:], recipf[:])
                    nc.vector.tensor_mul(attn_n[h][:, qsl], acc[:], rbs[:])

            # ============== Phase C: o_proj (partial) ==============
            # Runs inside the phase-B pool scope, reusing the scores psum
            # slots (same tag) and p-tile slots so no address-reuse barrier
            # separates the phases.
            for qc in range(NSC):
                qsl = slice(qc * SC, (qc + 1) * SC)
                for mt in range(DM // 128):
                    msl = slice(mt * 128, (mt + 1) * 128)
                    ops = psS.tile([128, SC], F32, tag="sc",
                                   name=f"ops{mt}_{qc}")
                    for h in range(HPC):
                        nc.tensor.matmul(ops[:], wo_t[h][:, msl],
                                         attn_n[h][:, qsl],
                                         start=(h == 0), stop=(h == HPC - 1))
                    osb = ppool.tile([128, SC], F32, tag="pt",
                                     name=f"osb{mt}_{qc}")
                    nc.vector.tensor_copy(osb[:], ops[:])
                    nc.sync.dma_start(out_T[msl, qsl], osb[:])

    nc.compile()
    return nc


def _host_inputs(hidden_states, qkv_w, o_w):
    """Build the 8 per-core input maps (sharding + layout transforms)."""
    # Head-dim permutation (shared by q and k; scores are invariant): RoPE
    # pair i=16b+j lands at partitions 32b+j (even) and 32b+16+j (odd), so the
    # pair swap is a within-32-block 16-rotation (one DVE stream_shuffle), with
    # the sign carried by the sin table: rot = x*cos2 + shuffle16(x)*sin2.
    inv_freq = 1.0 / (ROPE_THETA ** (np.arange(0, DH, 2, dtype=np.float32) / DH))
    t = np.arange(S, dtype=np.float32)
    ang = np.outer(inv_freq, t)                       # [64, S]
    cosv, sinv = np.cos(ang), np.sin(ang)
    cos_t = np.zeros((DH, S), dtype=np.float32)
    sin_t = np.zeros((DH, S), dtype=np.float32)
    perm = np.zeros(DH, dtype=np.int64)
    for b in range(4):
        for j in range(16):
            i = 16 * b + j
            perm[32 * b + j] = 2 * i
            perm[32 * b + 16 + j] = 2 * i + 1
            cos_t[32 * b + j] = cosv[i]
            cos_t[32 * b + 16 + j] = cosv[i]
            sin_t[32 * b + j] = -sinv[i]
            sin_t[32 * b + 16 + j] = sinv[i]
    hperm = np.concatenate([g * DH + perm for g in range(HPC)])  # per-head blocks

    mask_wide = np.zeros((128, 384 + SC), dtype=np.float32)
    k_idx = np.arange(128)[:, None]
    m_idx = np.arange(384 + SC)[None, :]
    mask_wide[(m_idx - 384) >= k_idx] = 1.0

    ones_col = np.ones((128, 1), dtype=np.float32)
    ones_row = np.ones((1, 128), dtype=np.float32)

    in_maps = []
    for c in range(N_CORES):
        b = c // TP
        g = c % TP
        hs = slice(g * DHC, (g + 1) * DHC)   # rows of q/k/v blocks for this group
        x_T = np.ascontiguousarray(hidden_states[:, b, :].T)
        wq_T = np.ascontiguousarray(qkv_w[0 * DM:1 * DM][hs][hperm].T)
        wk_T = np.ascontiguousarray(qkv_w[1 * DM:2 * DM][hs][hperm].T)
        wv_T = np.ascontiguousarray(qkv_w[2 * DM:3 * DM][hs].T)
        wo_T = np.ascontiguousarray(o_w[:, hs].T)
        in_maps.append({
            "x_T": x_T, "wq_T": wq_T, "wk_T": wk_T, "wv_T": wv_T, "wo_T": wo_T,
            "cos_t": cos_t, "sin_t": sin_t,
            "mask_wide": mask_wide, "ones_col": ones_col,
        })
    return in_maps


def kernel(hidden_states, sequence_mask, qkv_w, o_w, _results_hook=None):
    hidden_states = np.asarray(hidden_states, dtype=np.float32)
    qkv_w = np.asarray(qkv_w, dtype=np.float32)
    o_w = np.asarray(o_w, dtype=np.float32)
    # sequence_mask is all-True for this problem shape (spec fill=ones).

    if "nc" not in _CACHE:
        _CACHE["nc"] = _build_program()
    nc = _CACHE["nc"]

    in_maps = _host_inputs(hidden_states, qkv_w, o_w)
    res = bass_utils.run_bass_kernel_spmd(
        nc, in_maps, core_ids=list(range(N_CORES)), trace=False)
    if _results_hook is not None:
        _results_hook(res)

    out = np.zeros((S, B, DM), dtype=np.float64)
    for c in range(N_CORES):
        b = c // TP
        out[:, b, :] += res.results[c]["out_T"].T.astype(np.float64)
    return out.astype(np.float32)


if __name__ == "__main__":
    rng = np.random.default_rng(0)
    hs = rng.standard_normal((S, B, DM), dtype=np.float32)
    sm = np.ones((B, S), dtype=bool)
    qw = (rng.standard_normal((3 * DM, DM), dtype=np.float32) * 0.02)
    ow = (rng.standard_normal((DM, DM), dtype=np.float32) * 0.02)
    o = kernel(hs, sm, qw, ow)
    print("out", o.shape, o.dtype, float(np.abs(o).mean()))



# revision 31
# speedup vs baseline: 1.1098x; 1.1098x over previous
"""Trainium2 Bass kernel for CausalSelfAttention (d_model=2048, 16 heads, s=2048, b=2).

Sharding: data-parallel over batch (2) x tensor-parallel over heads (4 groups
of 4 heads) = 8 cores.  Each core: qkv projection for its 4 heads, RoPE,
causal attention, partial o_proj (row-parallel); host sums 4 partials/batch.

v2: single fused pipeline.  Projection chunk s, attention for chunk s-1 and
o_proj for chunk s-2 are emitted interleaved so every engine stays busy under
the PE roofline (~287us of mandatory matmul rows):

  - all matmuls in bf16 (inputs rounded host-side; PSUM accumulates fp32).
    bf16 runs at the same PE rate as f32r but halves SBUF/DMA, so q/k/v for
    all 4 heads stay SBUF-resident (no DRAM staging round-trip) and there is
    no min-256 moving-dim constraint on the causal-diagonal tiles.
  - RoPE applied in fp32 (psum evac on Pool, shuffle/muls on DVE+Pool),
    output rounded to bf16.
  - softmax denominator: diagonal score tiles summed on the PE (ones^T p);
    full tiles accumulated elementwise on DVE/Pool into an f32 tile that a
    single f32r matmul reduces -- saves ~19us of PE rows vs per-tile ones
    matmuls.
  - exp activation table preloaded at t=0; av matmuls trail their score
    matmul by 2 steps so the 612ns exp latency never stalls the in-order PE.
  - PSUM: proj 2 + scores 2 (warmup shares) + av-acc 2 + den 1 + o_proj 1
    = 8 banks.
"""

import sys

import numpy as np

_TRN_REPO = "/opt/trn_rl_repo"
if _TRN_REPO not in sys.path:
    sys.path.insert(0, _TRN_REPO)

import ml_dtypes  # noqa: E402

import concourse.tile as tile  # noqa: E402
import concourse.mybir as mybir  # noqa: E402
from concourse import bacc, bass_utils  # noqa: E402

# Problem constants (hardcoded per the contract).
S = 2048          # sequence length
B = 2             # batch
DM = 2048         # d_model
NH = 16           # heads total
DH = 128          # head dim
ROPE_THETA = 10000.0

N_CORES = 8
TP = 4            # head-parallel groups
HPC = NH // TP    # heads per core = 4
DHC = HPC * DH    # head-dim per core = 512

SC = 512          # s-chunk
NSC = S // SC     # 4 chunks
KT = DM // 128    # contraction tiles for projections = 16
HKT = KT // 2

SCALE = 1.0 / float(np.sqrt(DH))
EXP_SHIFT = -25.0  # softmax computed as exp(score*scale - 25); shift cancels

F32 = mybir.dt.float32
F32R = mybir.dt.float32r
BF16 = mybir.dt.bfloat16
NPBF = ml_dtypes.bfloat16

WARMUP_MMS = 30   # 512-row dummies spanning the startup DMA gate
# full-tile den accumulation: tiles [0, split) chain on DVE into da_a,
# [split, n_full) chain on Pool into da_b -- two single-engine chains so no
# cross-engine semaphore hops serialize the fold matmul.
DEN_SPLIT = {1: 2, 2: 5, 3: 8}

_CACHE = {}


def _build_program():
    nc = bacc.Bacc("TRN2", target_bir_lowering=False, debug=False,
                   num_devices=N_CORES)

    # ---- I/O ----
    x_T = nc.dram_tensor("x_T", [DM, S], BF16, kind="ExternalInput")
    wq_T = nc.dram_tensor("wq_T", [DM, DHC], BF16, kind="ExternalInput")
    wk_T = nc.dram_tensor("wk_T", [DM, DHC], BF16, kind="ExternalInput")
    wv_T = nc.dram_tensor("wv_T", [DM, DHC], BF16, kind="ExternalInput")
    wo_T = nc.dram_tensor("wo_T", [DHC, DM], BF16, kind="ExternalInput")
    cos_t = nc.dram_tensor("cos_t", [DH, S], BF16, kind="ExternalInput")
    sin_t = nc.dram_tensor("sin_t", [DH, S], BF16, kind="ExternalInput")
    mask_t = nc.dram_tensor("mask_t", [128, 128], BF16, kind="ExternalInput")
    ones_bf_t = nc.dram_tensor("ones_bf_t", [128, 1], BF16, kind="ExternalInput")
    ones_fr_t = nc.dram_tensor("ones_fr_t", [128, 1], F32R, kind="ExternalInput")
    out_T = nc.dram_tensor("out_T", [DM, S], F32, kind="ExternalOutput")

    with tile.TileContext(nc) as tc:
      with (
          tc.tile_pool(name="wts", bufs=1) as wpool,      # weights + consts
          tc.tile_pool(name="qkv", bufs=1) as apool,      # q/k/v chunk tiles
          tc.tile_pool(name="xin", bufs=4) as xpool,      # x halves
          tc.tile_pool(name="rope", bufs=2) as rpool,     # rope scratch
          tc.tile_pool(name="pt", bufs=8) as ppool,       # exp(p) tiles
          tc.tile_pool(name="an", bufs=8) as anpool,      # normalized attn
          tc.tile_pool(name="da", bufs=2) as dapool,      # den accumulators
          tc.tile_pool(name="sm", bufs=2) as spool,       # recip / rbs
          tc.tile_pool(name="ob", bufs=4) as opool,       # o_proj staging
      ):
        # ---------------- persistent tiles ----------------
        wq_w = [wpool.tile([128, HKT * DHC], BF16, tag=f"wqw{i}",
                           name=f"wqw{i}") for i in range(2)]
        wk_w = wpool.tile([128, KT * DHC], BF16, tag="wkw")
        wv_w = wpool.tile([128, KT * DHC], BF16, tag="wvw")
        wo_t = [wpool.tile([128, DM], BF16, tag=f"wo{h}", name=f"wo{h}")
                for h in range(HPC)]
        cos_sb = wpool.tile([DH, S], BF16, tag="cos")
        sin_sb = wpool.tile([DH, S], BF16, tag="sin")
        mask_sb = wpool.tile([128, 128], BF16, tag="mask")
        ones_bf = wpool.tile([128, 1], BF16, tag="onesb")
        ones_fr = wpool.tile([128, 1], F32R, tag="onesf")
        bias_sb = wpool.tile([128, 1], F32, tag="bias")
        wu_d = wpool.tile([128, 512], BF16, tag="wud")

        # q/k: [dh, s-chunk] per (head, chunk); v: [s-block, dhc] per block
        qt = [[apool.tile([DH, SC], BF16, tag=f"q{h}_{sc}", name=f"q{h}_{sc}")
               for sc in range(NSC)] for h in range(HPC)]
        kt_ = [[apool.tile([DH, SC], BF16, tag=f"k{h}_{sc}", name=f"k{h}_{sc}")
                for sc in range(NSC)] for h in range(HPC)]
        vblk = [apool.tile([128, DHC], BF16, tag=f"v{i}", name=f"v{i}")
                for i in range(S // 128)]

        # ---------------- PSUM pools (stages 0-4; s5 swaps to psOZ) --------
        _ps_ctx = [tc.tile_pool(name=n, bufs=b, space="PSUM")
                   for n, b in [("psMM", 2), ("psSC", 2), ("psAC", 2),
                                ("psDN", 1), ("psOP", 1)]]
        psMM, psSC, psAC, psDN, psOP = [p.__enter__() for p in _ps_ctx]

        # ---------------- startup ----------------
        nc.gpsimd.memset(wu_d[:], 0.0)
        nc.vector.memset(bias_sb[:], EXP_SHIFT)
        wu_ps = psSC.tile([128, SC], F32, tag="sc", name="wu_ps")

        def _warmup(n):
            for _ in range(n):
                nc.tensor.matmul(wu_ps[:, 0:SC], wu_d[:, 0:128],
                                 wu_d[:, 0:SC], start=True, stop=True)

        def _w_src(t, i0, n):
            return t[:, :].rearrange("(dk p) c -> p dk c", p=128)[:, i0:i0 + n, :]

        def _load_x_half(sc, i):
            xt = xpool.tile([128, HKT * SC], BF16, tag="x", name=f"x{sc}_{i}")
            src = (x_T[:, sc * SC:(sc + 1) * SC]
                   .rearrange("(dk p) s -> p dk s", p=128)[:, i * HKT:(i + 1) * HKT, :])
            nc.sync.dma_start(xt[:].rearrange("p (dk s) -> p dk s", s=SC), src)
            return xt

        def _wq_ap(dk):
            return wq_w[dk // HKT][:, (dk % HKT) * DHC:(dk % HKT + 1) * DHC]

        def _wk_ap(dk):
            return wk_w[:, dk * DHC:(dk + 1) * DHC]

        def _wv_ap(dk):
            return wv_w[:, dk * DHC:(dk + 1) * DHC]

        _warmup(WARMUP_MMS)
        # preload the Exp activation table off the critical path
        dummy = wpool.tile([128, 1], F32, tag="dumm")
        nc.scalar.activation(dummy[:], bias_sb[:],
                             mybir.ActivationFunctionType.Exp)

        nc.sync.dma_start(mask_sb[:], mask_t[:, :])
        nc.sync.dma_start(ones_bf[:], ones_bf_t[:, :])
        nc.sync.dma_start(ones_fr[:], ones_fr_t[:, :])
        x_half = {}
        x_half[(0, 0)] = _load_x_half(0, 0)
        nc.sync.dma_start(
            wq_w[0][:].rearrange("p (dk c) -> p dk c", c=DHC),
            _w_src(wq_T, 0, HKT))
        nc.sync.dma_start(
            wq_w[1][:].rearrange("p (dk c) -> p dk c", c=DHC),
            _w_src(wq_T, HKT, HKT))
        x_half[(0, 1)] = _load_x_half(0, 1)
        nc.sync.dma_start(
            wk_w[:].rearrange("p (dk c) -> p dk c", c=DHC),
            _w_src(wk_T, 0, KT))
        nc.sync.dma_start(cos_sb[:], cos_t[:, :])
        nc.sync.dma_start(sin_sb[:], sin_t[:, :])
        nc.sync.dma_start(
            wv_w[:].rearrange("p (dk c) -> p dk c", c=DHC),
            _w_src(wv_T, 0, KT))
        x_half[(1, 0)] = _load_x_half(1, 0)
        x_half[(1, 1)] = _load_x_half(1, 1)
        for hh in range(HPC):
            nc.sync.dma_start(wo_t[hh][:], wo_T[hh * 128:(hh + 1) * 128, :])

        # ---------------- op generators ----------------
        den_cnt = [0]

        def proj_fillers(sc):
            """Yield closures, one per PE matmul, for projection chunk sc.
            Chain-end closures also emit the evac/rope bundle."""
            xc = [x_half[(sc, dk // HKT)][:, (dk % HKT) * SC:(dk % HKT + 1) * SC]
                  for dk in range(KT)]
            ssl = slice(sc * SC, (sc + 1) * SC)
            chains = []
            for which, wap in (("q", _wq_ap), ("k", _wk_ap)):
                for h in range(HPC):
                    chains.append((which, h))
            for st in range(SC // 128):
                chains.append(("v", st))

            for which, idx in chains:
                hold = {}
                for dk in range(KT):
                    def mm(hold=hold, dk=dk, which=which, idx=idx, xc=xc,
                           sc=sc):
                        if dk == 0:
                            hold["ps"] = psMM.tile(
                                [128, SC], F32, tag="mm",
                                name=f"mm_{which}{idx}_{sc}")
                        ps = hold["ps"]
                        if which == "v":
                            nc.tensor.matmul(
                                ps[:], xc[dk][:, idx * 128:(idx + 1) * 128],
                                _wv_ap(dk), start=(dk == 0), stop=(dk == KT - 1))
                        else:
                            wap = _wq_ap if which == "q" else _wk_ap
                            hsl = slice(idx * DH, (idx + 1) * DH)
                            nc.tensor.matmul(
                                ps[:], wap(dk)[:, hsl], xc[dk],
                                start=(dk == 0), stop=(dk == KT - 1))
                    if dk < KT - 1:
                        yield mm
                        continue

                    def tail(mm=mm, hold=hold, which=which, idx=idx, sc=sc,
                             ssl=ssl):
                        mm()
                        ps = hold["ps"]
                        if which == "v":
                            nc.vector.tensor_copy(vblk[sc * 4 + idx][:], ps[:])
                            return
                        raw = rpool.tile([128, SC], F32, tag="raw")
                        nc.scalar.copy(raw[:], ps[:])
                        qsw = rpool.tile([128, SC], F32, tag="qsw")
                        nc.vector.stream_shuffle(
                            qsw[:], raw[:],
                            mask=list(range(16, 32)) + list(range(0, 16)))
                        nc.vector.tensor_mul(qsw[:], qsw[:], sin_sb[:, ssl])
                        t1 = rpool.tile([128, SC], F32, tag="t1")
                        nc.gpsimd.tensor_mul(t1[:], raw[:], cos_sb[:, ssl])
                        dst = (qt if which == "q" else kt_)[idx][sc]
                        nc.vector.tensor_add(dst[:], qsw[:], t1[:])
                    yield tail

        def attn_steps(qc):
            """Yield closures for attention of all heads at query chunk qc.
            Each step: score(kt) + exp + mask/den work + av(kt-2)."""
            qsl_lo = qc * SC
            n_kt = 4 * qc + 4
            n_full = n_kt - 4
            for h in range(HPC):
                u = {"pts": [None] * n_kt, "den_started": False}

                def av(kt, h=h, u=u, n_kt=n_kt, qc=qc):
                    off = kt * 128 - qc * SC
                    q_lo = max(0, off)
                    ln = SC - q_lo
                    nc.tensor.matmul(
                        u["acc"][:, q_lo:SC],
                        vblk[kt][:, h * DH:(h + 1) * DH],
                        u["pts"][kt][:, 0:ln],
                        start=(kt == 0), stop=(kt == n_kt - 1))

                for kt in range(n_kt):
                    def step(kt=kt, h=h, u=u, n_kt=n_kt, n_full=n_full,
                             qc=qc, av=av):
                        if kt == 0:
                            u["acc"] = psAC.tile([128, SC], F32, tag="ac",
                                                 name=f"ac{h}_{qc}")
                        off = kt * 128 - qc * SC
                        q_lo = max(0, off)
                        ln = SC - q_lo
                        sp = psSC.tile([128, SC], F32, tag="sc",
                                       name=f"sp{h}_{qc}_{kt}")
                        nc.tensor.matmul(
                            sp[:, 0:ln], kt_[h][kt // 4][:, (kt % 4) * 128:
                                                         (kt % 4 + 1) * 128],
                            qt[h][qc][:, q_lo:SC], start=True, stop=True)
                        if kt >= 2:
                            av(kt - 2)
                        pt = ppool.tile([128, SC], BF16, tag="pt",
                                        name=f"pt{h}_{qc}_{kt}")
                        u["pts"][kt] = pt
                        nc.scalar.activation(
                            pt[:, 0:ln], sp[:, 0:ln],
                            mybir.ActivationFunctionType.Exp,
                            bias=bias_sb[:], scale=SCALE)
                        if off >= 0:
                            # diagonal tile: mask (den matmul deferred to fin)
                            nc.vector.tensor_mul(
                                pt[:, 0:128], pt[:, 0:128], mask_sb[:, :])
                        else:
                            # full tile: single-engine accumulator chains
                            split = DEN_SPLIT[qc]
                            grp, eng, i0 = (
                                ("daa", nc.vector, 0) if kt < split
                                else ("dab", nc.gpsimd, split))
                            if kt == i0:
                                pass  # init pairs with the next tile
                            elif kt == i0 + 1:
                                u[grp] = dapool.tile([128, SC], F32R, tag=grp,
                                                     name=f"{grp}{h}_{qc}")
                                eng.tensor_add(u[grp][:],
                                               u["pts"][i0][:], pt[:])
                            else:
                                eng.tensor_add(u[grp][:], u[grp][:], pt[:])
                    yield step

                def fin(h=h, qc=qc, u=u, n_kt=n_kt, n_full=n_full, av=av):
                    # All den matmuls land here, so the single den psum bank
                    # is held for ~one unit less (no stall on the previous
                    # unit's reciprocal read).
                    den = psDN.tile([1, SC], F32, tag="dn", name=f"dn{h}_{qc}")
                    started = False
                    for grp in ("daa", "dab"):
                        if grp in u:
                            nc.tensor.matmul(
                                den[:, 0:SC], ones_fr[:], u[grp][:],
                                start=(not started), stop=False)
                            started = True
                    for kt in range(n_full, n_kt):
                        q_lo = kt * 128 - qc * SC
                        nc.tensor.matmul(
                            den[:, q_lo:SC], ones_bf[:],
                            u["pts"][kt][:, 0:SC - q_lo],
                            start=(not started), stop=(kt == n_kt - 1))
                        started = True
                    av(n_kt - 2)
                    av(n_kt - 1)
                    recipf = spool.tile([1, SC], F32, tag="recipf")
                    nc.vector.reciprocal_approx_fast(out=recipf[:],
                                                     in_=den[:])
                    rbs = spool.tile([128, SC], F32, tag="rbs")
                    nc.gpsimd.partition_broadcast(rbs[:], recipf[:])
                    an = anpool.tile([DH, SC], BF16, tag="an",
                                     name=f"an{h}_{qc}")
                    attn_n[h][qc] = an
                    nc.vector.tensor_mul(an[:], u["acc"][:], rbs[:])
                yield fin

        attn_n = [[None] * NSC for _ in range(HPC)]

        def oproj_fillers(qc, pspool=None, pstag="op"):
            """Yield per-matmul closures for o_proj of chunk qc."""
            if pspool is None:
                pspool = psOP
            qsl = slice(qc * SC, (qc + 1) * SC)
            for mt in range(DM // 128):
                msl = slice(mt * 128, (mt + 1) * 128)
                hold = {}
                for h in range(HPC):
                    def mm(h=h, hold=hold, msl=msl, qc=qc, mt=mt,
                           pspool=pspool, pstag=pstag):
                        if h == 0:
                            hold["ops"] = pspool.tile([128, SC], F32,
                                                      tag=pstag,
                                                      name=f"ops{mt}_{qc}")
                        nc.tensor.matmul(hold["ops"][:], wo_t[h][:, msl],
                                         attn_n[h][qc][:], start=(h == 0),
                                         stop=(h == HPC - 1))
                    if h < HPC - 1:
                        yield mm
                        continue

                    def tail(mm=mm, hold=hold, mt=mt, qc=qc, msl=msl, qsl=qsl):
                        mm()
                        osb = opool.tile([128, SC], F32, tag="ob",
                                         name=f"osb{mt}_{qc}")
                        if qc != 2:  # Act has slack except mid-qc3-exp (s4)
                            nc.scalar.copy(osb[:], hold["ops"][:])
                        else:
                            nc.vector.tensor_copy(osb[:], hold["ops"][:])
                        nc.sync.dma_start(out_T[msl, qsl], osb[:])
                    yield tail

        # ---------------- interleaved emission ----------------
        def emit_stage(attn_qc, proj_sc, oproj_qc, prefetch_sc,
                       tail_frac=0.0):
            """Emit one pipeline stage.  Attention steps are spread through
            the first (1-tail_frac) of the filler list so their finalize
            chains drain under the remaining fillers."""
            fillers = []
            if prefetch_sc is not None and prefetch_sc < NSC:
                def pf(prefetch_sc=prefetch_sc):
                    x_half[(prefetch_sc, 0)] = _load_x_half(prefetch_sc, 0)
                    x_half[(prefetch_sc, 1)] = _load_x_half(prefetch_sc, 1)
                fillers.append(pf)
            if proj_sc is not None:
                fillers.extend(proj_fillers(proj_sc))
            if oproj_qc is not None:
                fillers.extend(oproj_fillers(oproj_qc))
            steps = list(attn_steps(attn_qc)) if attn_qc is not None else []
            if not steps:
                for f in fillers:
                    f()
                return
            nf, ns = len(fillers), len(steps)
            spread_n = min(nf, max(ns, int(nf * (1.0 - tail_frac))))
            fi = 0
            for si, st in enumerate(steps):
                st()
                target = (si + 1) * spread_n // ns
                while fi < target:
                    fillers[fi]()
                    fi += 1
            while fi < nf:
                fillers[fi]()
                fi += 1

        emit_stage(None, 0, None, 2)      # s0: proj(0), prefetch x2
        emit_stage(0, 1, None, 3, 0.3)    # s1: attn(0) | proj(1)
        emit_stage(1, 2, 0, None, 0.3)    # s2: attn(1) | proj(2) | o_proj(0)
        emit_stage(2, 3, 1, None, 0.35)   # s3: attn(2) | proj(3) | o_proj(1)
        emit_stage(3, None, 2, None, 0.45)  # s4: attn(3) | o_proj(2)

        # s5: o_proj(3) with a deeper psum rotation (other pools closed)
        for p in reversed(_ps_ctx):
            p.__exit__(None, None, None)
        with tc.tile_pool(name="psOZ", bufs=4, space="PSUM") as psOZ:
            for f in oproj_fillers(3, pspool=psOZ, pstag="oz"):
                f()

    nc.compile()
    return nc


def _host_inputs(hidden_states, qkv_w, o_w):
    """Build the 8 per-core input maps (sharding + layout transforms)."""
    # RoPE pair i=16b+j lands at partitions 32b+j (even) and 32b+16+j (odd):
    # the pair swap is a within-32-block 16-rotation (one DVE stream_shuffle),
    # with the sign carried by the sin table.
    inv_freq = 1.0 / (ROPE_THETA ** (np.arange(0, DH, 2, dtype=np.float32) / DH))
    t = np.arange(S, dtype=np.float32)
    ang = np.outer(inv_freq, t)                       # [64, S]
    cosv, sinv = np.cos(ang), np.sin(ang)
    cos_t = np.zeros((DH, S), dtype=np.float32)
    sin_t = np.zeros((DH, S), dtype=np.float32)
    perm = np.zeros(DH, dtype=np.int64)
    for b in range(4):
        for j in range(16):
            i = 16 * b + j
            perm[32 * b + j] = 2 * i
            perm[32 * b + 16 + j] = 2 * i + 1
            cos_t[32 * b + j] = cosv[i]
            cos_t[32 * b + 16 + j] = cosv[i]
            sin_t[32 * b + j] = -sinv[i]
            sin_t[32 * b + 16 + j] = sinv[i]
    cos_t = cos_t.astype(NPBF)
    sin_t = sin_t.astype(NPBF)
    hperm = np.concatenate([g * DH + perm for g in range(HPC)])

    # mask128[p, j] = 1 if j >= p (valid) else 0, for diagonal score tiles
    mask128 = (np.arange(128)[None, :] >= np.arange(128)[:, None])
    mask128 = mask128.astype(NPBF)
    ones_bf = np.ones((128, 1), dtype=NPBF)
    ones_fr = np.ones((128, 1), dtype=np.float32)

    in_maps = []
    for c in range(N_CORES):
        b = c // TP
        g = c % TP
        hs = slice(g * DHC, (g + 1) * DHC)
        x_T = np.ascontiguousarray(hidden_states[:, b, :].T).astype(NPBF)
        wq_T = np.ascontiguousarray(qkv_w[0 * DM:1 * DM][hs][hperm].T).astype(NPBF)
        wk_T = np.ascontiguousarray(qkv_w[1 * DM:2 * DM][hs][hperm].T).astype(NPBF)
        wv_T = np.ascontiguousarray(qkv_w[2 * DM:3 * DM][hs].T).astype(NPBF)
        wo_T = np.ascontiguousarray(o_w[:, hs].T).astype(NPBF)
        in_maps.append({
            "x_T": x_T, "wq_T": wq_T, "wk_T": wk_T, "wv_T": wv_T, "wo_T": wo_T,
            "cos_t": cos_t, "sin_t": sin_t,
            "mask_t": mask128, "ones_bf_t": ones_bf, "ones_fr_t": ones_fr,
        })
    return in_maps


def kernel(hidden_states, sequence_mask, qkv_w, o_w, _results_hook=None):
    hidden_states = np.asarray(hidden_states, dtype=np.float32)
    qkv_w = np.asarray(qkv_w, dtype=np.float32)
    o_w = np.asarray(o_w, dtype=np.float32)
    # sequence_mask is all-True for this problem shape (spec fill=ones).

    if "nc" not in _CACHE:
        _CACHE["nc"] = _build_program()
    nc = _CACHE["nc"]

    in_maps = _host_inputs(hidden_states, qkv_w, o_w)
    res = bass_utils.run_bass_kernel_spmd(
        nc, in_maps, core_ids=list(range(N_CORES)), trace=False)
    if _results_hook is not None:
        _results_hook(res)

    out = np.zeros((S, B, DM), dtype=np.float64)
    for c in range(N_CORES):
        b = c // TP
        out[:, b, :] += res.results[c]["out_T"].T.astype(np.float64)
    return out.astype(np.float32)


if __name__ == "__main__":
    rng = np.random.default_rng(0)
    hs = rng.standard_normal((S, B, DM), dtype=np.float32)
    sm = np.ones((B, S), dtype=bool)
    qw = (rng.standard_normal((3 * DM, DM), dtype=np.float32) * 0.02)
    ow = (rng.standard_normal((DM, DM), dtype=np.float32) * 0.02)
    o = kernel(hs, sm, qw, ow)
    print("out", o.shape, o.dtype, float(np.abs(o).mean()))
